# revision 17
# baseline (speedup 1.0000x reference)
"""GAT block kernel for Trainium2 (8 NeuronCores, data-parallel over batch).

Math (per batch b, frame f, head h; n=64 nodes, d=16 head dim):
  h_feat = x^T @ W1 + b1                                   # [n, 64]
  s_src[i] = h_feat[i, h*16:(h+1)*16] . W2[:16,0]
  s_dst[j] = h_feat[j, h*16:(h+1)*16] . W2[16:,0]
  scores[i,j] = lrelu(s_src[i]+s_dst[j]+b2) * mask[i,j]
  attn = softmax_j(scores); out[:,i,:] = attn[i,i] * h_feat[i,:]

Key identities used:
  * s_src/s_dst are tiny linear maps of x directly (fold W1/W2) -> no need to
    materialize h_feat for the scores.
  * final einsum 'bhfnn,bhfnj->bhfnj' takes only diag(attn): out = attn_ii * h.
  * mask >= 0  =>  mask * lrelu(z) == lrelu(mask * z)
  * softmax diag: attn_ii = exp(s_ii) / sum_j exp(s_ij)  (scores are O(1), so
    skipping max-subtraction is exact up to fp rounding)
  * out = attn*(x@W1+b1) = (x*attn)@W1 + attn*b1 -> one K=16 matmul with the
    per-head attn folded into an augmented rhs.

Layouts:
  * scores keep f (frames) on the 128 partitions and (i, j) on the free dim so
    BOTH broadcast operands (src over j, dst over i) are stride-0 free-dim APs.
  * the src/dst projection runs as a K=96 matmul (c x 32 node block) against a
    host-built block-delta weight so PSUM comes out as [f, (head, node)]
    directly; the PSUM->SBUF move is fused with the bias add.
"""

import numpy as np

B, C, F, N = 16, 3, 512, 64
H, D = 4, 16
NCORES = 8
BPC = B // NCORES  # batches per core
SLOPE = 0.01
IG = 32  # node-block size for the K=96 projection matmul
LRELU = True  # False -> plain Relu (for CoreSim, which lacks Lrelu)

_CACHE = {}


def _build_nc():
    import concourse.bass as bass
    import concourse.bacc as bacc
    import concourse.tile as tile
    from concourse import mybir

    AF = mybir.ActivationFunctionType
    ALU = mybir.AluOpType
    AX = mybir.AxisListType
    dt = mybir.dt.float32
    AP = bass.AP

    nc = bacc.Bacc(None, target_bir_lowering=False)

    xc = nc.dram_tensor("xc", [BPC, C, F, N], dt, kind="ExternalInput")
    xtc = nc.dram_tensor("xtc", [BPC, C, N, F], dt, kind="ExternalInput")
    maskf = nc.dram_tensor("maskf", [N, N], dt, kind="ExternalInput")
    wdel = nc.dram_tensor("wdel", [C * IG, 2 * H * IG], dt, kind="ExternalInput")
    vall = nc.dram_tensor("vall", [1, 2 * H * N], dt, kind="ExternalInput")
    wout = nc.dram_tensor("wout", [2 * (H * C + H), 2 * H * D], dt, kind="ExternalInput")
    out_c = nc.dram_tensor("out_c", [BPC, H * D, F, N], dt, kind="ExternalOutput")

    # DRAM strides (elements)
    XS_B, XS_C, XS_F = C * F * N, F * N, N
    XT_B, XT_C, XT_N = C * N * F, N * F, F
    OS_B, OS_K, OS_F = H * D * F * N, F * N, N

    def rap(t, off, dims):
        a = t[:]
        return AP(tensor=a.tensor, offset=a.offset + off, ap=dims)

    def pitch(t):
        return t[:].ap[0][0]

    with tile.TileContext(nc) as tc:
        with (
            tc.tile_pool(name="singles", bufs=1) as singles,
            tc.tile_pool(name="xT", bufs=2) as xT_pool,
            tc.tile_pool(name="sdv", bufs=2) as sdv_pool,
            tc.tile_pool(name="zt", bufs=2) as z_pool,
            tc.tile_pool(name="wt", bufs=2) as w_pool,
            tc.tile_pool(name="den", bufs=2) as den_pool,
            tc.tile_pool(name="small", bufs=4) as small,
            tc.tile_pool(name="attn", bufs=2) as attn_pool,
            tc.tile_pool(name="xt", bufs=2) as xt_pool,
            tc.tile_pool(name="prod", bufs=2) as prod_pool,
            tc.tile_pool(name="rhs", bufs=2) as rhs_pool,
            tc.tile_pool(name="stage", bufs=3) as stage_pool,
            tc.tile_pool(name="ps_s", bufs=2, space="PSUM") as psum_s_pool,
            tc.tile_pool(name="ps_o", bufs=3, space="PSUM") as psum_o_pool,
        ):
            # ---- one-time setup ----
            mask_rep = singles.tile([128, N * N], dt)
            nc.sync.dma_start(out=mask_rep[:], in_=rap(maskf, 0, [[0, 128], [1, N * N]]))
            v_rep = singles.tile([128, 2 * H * N], dt)
            nc.sync.dma_start(out=v_rep[:], in_=rap(vall, 0, [[0, 128], [1, 2 * H * N]]))
            wdel_sb = singles.tile([C * IG, 2 * H * IG], dt)
            nc.sync.dma_start(out=wdel_sb[:], in_=wdel[:])
            wout_sb = singles.tile([2 * (H * C + H), 2 * H * D], dt)
            nc.sync.dma_start(out=wout_sb[:], in_=wout[:])
            zbias = singles.tile([128, 1], dt)
            nc.vector.memset(zbias[:], 0.0)

            for b in range(BPC):
                # lhsT for the projection: [(c, i_local), f] per node-group
                xTg = []
                for g in range(N // IG):
                    xg = xT_pool.tile([C * IG, F], dt)
                    nc.sync.dma_start(
                        out=xg[:],
                        in_=rap(xtc, b * XT_B + g * IG * XT_N,
                                [[XT_C, C], [XT_N, IG], [1, F]]),
                    )
                    xTg.append(xg)

                for ch in range(4):  # chunks of 128 frames
                    f0 = ch * 128
                    # ---- projection: psum8[f, (g, hcol, i32)] ----
                    ps = psum_s_pool.tile([128, 2 * H * N], dt)
                    for g in range(N // IG):
                        nc.tensor.matmul(
                            ps[:, g * (2 * H * IG):(g + 1) * (2 * H * IG)],
                            xTg[g][:, f0:f0 + 128],
                            wdel_sb[:],
                            start=True, stop=True,
                        )
                    # ---- sd_v[f, (hcol, i)] = psum(permuted) + v ----
                    sd_v = sdv_pool.tile([128, 2 * H * N], dt)
                    pp = pitch(ps)
                    sp = pitch(sd_v)
                    vp = pitch(v_rep)
                    nc.vector.tensor_add(
                        rap(sd_v, 0, [[sp, 128], [N, 2 * H], [IG, 2], [1, IG]]),
                        rap(ps, 0, [[pp, 128], [IG, 2 * H], [2 * H * IG, 2], [1, IG]]),
                        rap(v_rep, 0, [[vp, 128], [N, 2 * H], [IG, 2], [1, IG]]),
                    )

                    denom = den_pool.tile([128, H * N], dt)
                    for h in range(H):
                        # z[f, i, j] = src[f,i] + dst[f,j]
                        z = z_pool.tile([128, N * N], dt)
                        nc.vector.tensor_add(
                            z[:],
                            rap(sd_v, h * N, [[sp, 128], [1, N], [0, N]]),
                            rap(sd_v, H * N + h * N, [[sp, 128], [0, N], [1, N]]),
                        )
                        # w = mask * z   (gpsimd, to offload DVE)
                        w = w_pool.tile([128, N * N], dt)
                        nc.gpsimd.tensor_mul(w[:], z[:], mask_rep[:])
                        # E = exp(lrelu(w)) in place on ACT
                        if LRELU:
                            nc.scalar.activation(w[:], w[:], AF.Lrelu, bias=zbias[:], alpha=SLOPE)
                        else:
                            nc.scalar.activation(w[:], w[:], AF.Relu, bias=zbias[:])
                        nc.scalar.activation(w[:], w[:], AF.Exp, bias=zbias[:])
                        wp = pitch(w)
                        nc.vector.tensor_reduce(
                            denom[:, h * N:(h + 1) * N],
                            rap(w, 0, [[wp, 128], [N, N], [1, N]]),
                            axis=AX.X, op=ALU.add,
                        )

                    # ---- diagonal numerator and attn = E_d / denom ----
                    sdd = small.tile([128, H * N], dt)
                    nc.vector.tensor_add(sdd[:], sd_v[:, 0:H * N], sd_v[:, H * N:2 * H * N])
                    mp = pitch(mask_rep)
                    nc.vector.tensor_mul(
                        sdd[:], sdd[:], rap(mask_rep, 0, [[mp, 128], [0, H], [N + 1, N]])
                    )
                    wdd = small.tile([128, H * N], dt)
                    if LRELU:
                        nc.scalar.activation(wdd[:], sdd[:], AF.Lrelu, bias=zbias[:], alpha=SLOPE)
                    else:
                        nc.scalar.activation(wdd[:], sdd[:], AF.Relu, bias=zbias[:])
                    nc.scalar.activation(wdd[:], wdd[:], AF.Exp, bias=zbias[:])
                    rec = small.tile([128, H * N], dt)
                    nc.vector.reciprocal(rec[:], denom[:])
                    attn = attn_pool.tile([128, H * N], dt)
                    nc.vector.tensor_mul(attn[:], wdd[:], rec[:])

                    # ---- prod[f, (h,c,n)] = x[f,(c,n)] * attn[f,(h,n)] ----
                    xt = xt_pool.tile([128, C * N], dt)
                    nc.sync.dma_start(
                        out=xt[:],
                        in_=rap(xc, b * XS_B + f0 * XS_F, [[XS_F, 128], [XS_C, C], [1, N]]),
                    )
                    prod = prod_pool.tile([128, H * C * N], dt)
                    xp_, ap_ = pitch(xt), pitch(attn)
                    nc.vector.tensor_mul(
                        prod[:],
                        rap(xt, 0, [[xp_, 128], [0, H], [N, C], [1, N]]),
                        rap(attn, 0, [[ap_, 128], [N, H], [0, C], [1, N]]),
                    )

                    # ---- gather to rhs32 rows (two 32-frame groups stacked)
                    #      and K=32 block-diag matmul -> [128=(mh,k), 512] ----
                    ppr = pitch(prod)
                    R = H * C + H  # 16
                    for hf in range(2):  # 64 frames each
                        rhs = rhs_pool.tile([2 * R, 32 * N], dt)
                        for r in range(H * C):
                            nc.sync.dma_start(
                                out=rhs[2 * r:2 * r + 2, :],
                                in_=rap(prod, hf * 64 * ppr + r * N, [[ppr, 64], [1, N]]),
                            )
                        for h in range(H):
                            r = H * C + h
                            nc.sync.dma_start(
                                out=rhs[2 * r:2 * r + 2, :],
                                in_=rap(attn, hf * 64 * ap_ + h * N, [[ap_, 64], [1, N]]),
                            )
                        for t in range(4):  # 8 frames per group per matmul
                            po = psum_o_pool.tile([128, 512], dt)
                            nc.tensor.matmul(
                                po[:], wout_sb[:], rhs[:, t * 512:(t + 1) * 512],
                                start=True, stop=True,
                            )
                            st = stage_pool.tile([128, 512], dt)
                            if t % 2 == 0:
                                nc.vector.tensor_copy(st[:], po[:])
                            else:
                                nc.scalar.copy(st[:], po[:])
                            base = b * OS_B + (f0 + hf * 64 + t * 8) * OS_F
                            nc.sync.dma_start(
                                out=rap(out_c, base,
                                        [[32 * OS_F, 2], [OS_K, H * D], [OS_F, 8], [1, N]]),
                                in_=st[:],
                            )
    nc.compile()
    return nc


def _host_prep(x, mask, W1, b1, W2, b2):
    x = np.ascontiguousarray(np.asarray(x, dtype=np.float32))
    xt = np.ascontiguousarray(x.transpose(0, 1, 3, 2))  # [B, C, N, F]
    mask = np.ascontiguousarray(np.asarray(mask, dtype=np.float32))
    W1 = np.asarray(W1, dtype=np.float32)
    b1 = np.asarray(b1, dtype=np.float32)
    W2 = np.asarray(W2, dtype=np.float32)
    b2 = np.asarray(b2, dtype=np.float32)

    a_src, a_dst = W2[:D, 0], W2[D:, 0]
    W1h = W1.reshape(C, H, D)
    b1h = b1.reshape(H, D)
    mat3 = np.concatenate([W1h @ a_src, W1h @ a_dst], axis=1).astype(np.float32)  # [3, 8]
    v_src = (b1h @ a_src + b2[0]).astype(np.float32)  # [4] (b2 folded here)
    v_dst = (b1h @ a_dst).astype(np.float32)
    vall = np.concatenate([np.repeat(v_src, N), np.repeat(v_dst, N)])[None, :].astype(np.float32)

    # block-delta projection weight: [(c,i'), (hcol, i)] = mat3[c,hcol]*delta(i,i')
    wdel = np.zeros((C, IG, 2 * H, IG), dtype=np.float32)
    for c in range(C):
        for hc in range(2 * H):
            wdel[c, :, hc, :] = mat3[c, hc] * np.eye(IG, dtype=np.float32)
    wdel = wdel.reshape(C * IG, 2 * H * IG)

    wsmall = np.zeros((H * C + H, H * D), dtype=np.float32)
    for h in range(H):
        for c in range(C):
            wsmall[h * C + c, h * D:(h + 1) * D] = W1[c, h * D:(h + 1) * D]
        wsmall[H * C + h, h * D:(h + 1) * D] = b1[h * D:(h + 1) * D]
    # block-diagonal x2 so the out matmul fills all 128 PSUM partitions;
    # the two 32-frame groups (mh) are interleaved as rows 2r+mh
    R = H * C + H
    wout = np.zeros((2 * R, 2 * H * D), dtype=np.float32)
    for mh in range(2):
        wout[mh::2, mh * H * D:(mh + 1) * H * D] = wsmall

    return x, xt, mask, wdel, vall, wout


def _run(inputs, trace=False):
    from concourse.bass_utils import run_bass_kernel_spmd

    x, xt, mask, wdel, vall, wout = _host_prep(
        inputs["x"], inputs["mask"], inputs["W1"], inputs["b1"],
        inputs["W2"], inputs["b2"],
    )
    if "nc" not in _CACHE:
        _CACHE["nc"] = _build_nc()
    nc = _CACHE["nc"]

    in_maps = []
    for c in range(NCORES):
        in_maps.append({
            "xc": np.ascontiguousarray(x[c * BPC:(c + 1) * BPC]),
            "xtc": np.ascontiguousarray(xt[c * BPC:(c + 1) * BPC]),
            "maskf": mask,
            "wdel": wdel,
            "vall": vall,
            "wout": wout,
        })
    res = run_bass_kernel_spmd(nc, in_maps, core_ids=list(range(NCORES)), trace=trace)
    out = np.concatenate([r["out_c"] for r in res.results], axis=0)
    return out, res


def kernel(**inputs):
    out, _ = _run(inputs, trace=False)
    return out


if __name__ == "__main__":
    rng = np.random.default_rng(0)
    ins = {
        "x": rng.standard_normal((B, C, F, N), dtype=np.float32),
        "mask": rng.random((N, N), dtype=np.float32),
        "W1": 0.1 * rng.standard_normal((C, H * D), dtype=np.float32),
        "b1": 0.1 * rng.standard_normal((H * D,), dtype=np.float32),
        "W2": 0.1 * rng.standard_normal((2 * D, 1), dtype=np.float32),
        "b2": 0.1 * rng.standard_normal((1,), dtype=np.float32),
    }
    out = kernel(**ins)
    print(out.shape, out.dtype)


# revision 33
# speedup vs baseline: 8061.6787x; 8061.6787x over previous
"""GAT block kernel for Trainium2 (8 NeuronCores, data-parallel over batch).

Math (per batch b, frame f, head h; n=64 nodes, d=16 head dim):
  h_feat = x^T @ W1 + b1                                   # [n, 64]
  s_src[i] = h_feat[i, h*16:(h+1)*16] . W2[:16,0]
  s_dst[j] = h_feat[j, h*16:(h+1)*16] . W2[16:,0]
  scores[i,j] = lrelu(s_src[i]+s_dst[j]+b2) * mask[i,j]
  attn = softmax_j(scores); out[:,i,:] = attn[i,i] * h_feat[i,:]

Key identities used:
  * s_src/s_dst are tiny linear maps of x directly (fold W1/W2) -> no need to
    materialize h_feat for the scores.
  * final einsum 'bhfnn,bhfnj->bhfnj' takes only diag(attn): out = attn_ii * h.
  * mask >= 0  =>  mask * lrelu(z) == lrelu(mask * z)
  * softmax diag: attn_ii = exp(s_ii) / sum_j exp(s_ij)  (scores are O(1), so
    skipping max-subtraction is exact up to fp rounding)
  * out = attn*(x@W1+b1) = (x*attn)@W1 + attn*b1 -> one K=16 matmul with the
    per-head attn folded into an augmented rhs.

Layouts:
  * scores keep f (frames) on the 128 partitions and (i, j) on the free dim so
    BOTH broadcast operands (src over j, dst over i) are stride-0 free-dim APs.
  * the src/dst projection runs as a K=96 matmul (c x 32 node block) against a
    host-built block-delta weight so PSUM comes out as [f, (head, node)]
    directly; the PSUM->SBUF move is fused with the bias add.
"""

import numpy as np

B, C, F, N = 16, 3, 512, 64
H, D = 4, 16
NCORES = 8
BPC = B // NCORES  # batches per core
SLOPE = 0.01
IG = 32  # node-block size for the K=96 projection matmul
LRELU = True  # False -> plain Relu (for CoreSim, which lacks Lrelu)

_CACHE = {}


def _build_nc():
    import concourse.bass as bass
    import concourse.bacc as bacc
    import concourse.tile as tile
    from concourse import mybir

    AF = mybir.ActivationFunctionType
    ALU = mybir.AluOpType
    AX = mybir.AxisListType
    dt = mybir.dt.float32
    dt16 = mybir.dt.float16
    AP = bass.AP

    nc = bacc.Bacc(None, target_bir_lowering=False)

    xc = nc.dram_tensor("xc", [BPC, C, F, N], dt, kind="ExternalInput")
    xtc = nc.dram_tensor("xtc", [BPC, C, N, F], dt, kind="ExternalInput")
    maskf = nc.dram_tensor("maskf", [N, N], dt16, kind="ExternalInput")
    mdiag = nc.dram_tensor("mdiag", [1, N], dt, kind="ExternalInput")
    wdel = nc.dram_tensor("wdel", [C * IG, 2 * H * IG], dt, kind="ExternalInput")
    vall = nc.dram_tensor("vall", [1, 2 * H * N], dt, kind="ExternalInput")
    wout = nc.dram_tensor("wout", [2 * (H * C + H), 2 * H * D], dt, kind="ExternalInput")
    out_c = nc.dram_tensor("out_c", [BPC, H * D, F, N], dt, kind="ExternalOutput")

    # DRAM strides (elements)
    XS_B, XS_C, XS_F = C * F * N, F * N, N
    XT_B, XT_C, XT_N = C * N * F, N * F, F
    OS_B, OS_K, OS_F = H * D * F * N, F * N, N

    def rap(t, off, dims):
        a = t[:]
        return AP(tensor=a.tensor, offset=a.offset + off, ap=dims)

    def pitch(t):
        return t[:].ap[0][0]

    with tile.TileContext(nc) as tc:
        with (
            tc.tile_pool(name="singles", bufs=1) as singles,
            tc.tile_pool(name="xT", bufs=4) as xT_pool,
            tc.tile_pool(name="sdv", bufs=3) as sdv_pool,
            tc.tile_pool(name="zt", bufs=3) as z_pool,
            tc.tile_pool(name="wt", bufs=4) as w_pool,
            tc.tile_pool(name="den", bufs=3) as den_pool,
            tc.tile_pool(name="small", bufs=4) as small,
            tc.tile_pool(name="attn", bufs=3) as attn_pool,
            tc.tile_pool(name="xt", bufs=8) as xt_pool,
            tc.tile_pool(name="prod", bufs=3) as prod_pool,
            tc.tile_pool(name="rhs", bufs=3) as rhs_pool,
            tc.tile_pool(name="stage", bufs=4) as stage_pool,
            tc.tile_pool(name="ps_s", bufs=3, space="PSUM") as psum_s_pool,
            tc.tile_pool(name="ps_o", bufs=2, space="PSUM") as psum_o_pool,
        ):
            # ---- one-time setup ----
            mask_rep = singles.tile([128, N * N], dt16)
            nc.sync.dma_start(out=mask_rep[:], in_=rap(maskf, 0, [[0, 128], [1, N * N]]))
            mdiag_rep = singles.tile([128, N], dt)
            nc.sync.dma_start(out=mdiag_rep[:], in_=rap(mdiag, 0, [[0, 128], [1, N]]))
            v_rep = singles.tile([128, 2 * H * N], dt)
            nc.sync.dma_start(out=v_rep[:], in_=rap(vall, 0, [[0, 128], [1, 2 * H * N]]))
            wdel_sb = singles.tile([C * IG, 2 * H * IG], dt)
            nc.sync.dma_start(out=wdel_sb[:], in_=wdel[:])
            wout_sb = singles.tile([2 * (H * C + H), 2 * H * D], dt)
            nc.sync.dma_start(out=wout_sb[:], in_=wout[:])
            zbias = singles.tile([128, 1], dt)
            nc.vector.memset(zbias[:], 0.0)

            # lhsT for the projection: [(c, i_local), f] per (b, node-group),
            # all loaded upfront so batch boundaries don't stall the SP queue
            xTgs = []
            for b in range(BPC):
                xTg = []
                for g in range(N // IG):
                    xg = xT_pool.tile([C * IG, F], dt)
                    nc.sync.dma_start(
                        out=xg[:],
                        in_=rap(xtc, b * XT_B + g * IG * XT_N,
                                [[XT_C, C], [XT_N, IG], [1, F]]),
                    )
                    xTg.append(xg)
                xTgs.append(xTg)
            xts = []
            for b in range(BPC):
                row = []
                for ch in range(4):
                    xt = xt_pool.tile([128, C * N], dt)
                    nc.sync.dma_start(
                        out=xt[:],
                        in_=rap(xc, b * XS_B + ch * 128 * XS_F,
                                [[XS_F, 128], [XS_C, C], [1, N]]),
                    )
                    row.append(xt)
                xts.append(row)

            for b in range(BPC):
                xTg = xTgs[b]
                for ch in range(4):  # chunks of 128 frames
                    f0 = ch * 128
                    # ---- projection: psum8[f, (g, hcol, i32)] ----
                    ps = psum_s_pool.tile([128, 2 * H * N], dt)
                    for g in range(N // IG):
                        nc.tensor.matmul(
                            ps[:, g * (2 * H * IG):(g + 1) * (2 * H * IG)],
                            xTg[g][:, f0:f0 + 128],
                            wdel_sb[:],
                            start=True, stop=True,
                        )
                    # ---- sd_v[f, (hcol, i)] = psum(permuted) + v ----
                    sd_v = sdv_pool.tile([128, 2 * H * N], dt)
                    pp = pitch(ps)
                    sp = pitch(sd_v)
                    vp = pitch(v_rep)
                    nc.vector.tensor_add(
                        rap(sd_v, 0, [[sp, 128], [N, 2 * H], [IG, 2], [1, IG]]),
                        rap(ps, 0, [[pp, 128], [IG, 2 * H], [2 * H * IG, 2], [1, IG]]),
                        rap(v_rep, 0, [[vp, 128], [N, 2 * H], [IG, 2], [1, IG]]),
                    )

                    denom = den_pool.tile([128, H * N], dt)
                    for h in range(H):
                        # z[f, i, j] = src[f,i] + dst[f,j]
                        z = z_pool.tile([128, N * N], dt16)
                        nc.gpsimd.tensor_add(
                            z[:],
                            rap(sd_v, h * N, [[sp, 128], [1, N], [0, N]]),
                            rap(sd_v, H * N + h * N, [[sp, 128], [0, N], [1, N]]),
                        )
                        # w = mask * z   (fp16 tensor_tensor: DVE 2x mode)
                        w = w_pool.tile([128, N * N], dt16)
                        nc.vector.tensor_mul(w[:], z[:], mask_rep[:])
                        # E = exp(lrelu(w)) in place on ACT
                        if LRELU:
                            nc.scalar.activation(w[:], w[:], AF.Prelu, bias=zbias[:], alpha=SLOPE)
                        else:
                            nc.scalar.activation(w[:], w[:], AF.Relu, bias=zbias[:])
                        nc.scalar.activation(w[:], w[:], AF.Exp, bias=zbias[:])
                        wp = pitch(w)
                        nc.vector.tensor_reduce(
                            denom[:, h * N:(h + 1) * N],
                            rap(w, 0, [[wp, 128], [N, N], [1, N]]),
                            axis=AX.X, op=ALU.add,
                        )

                    # ---- diagonal numerator and attn = E_d / denom ----
                    sdd = small.tile([128, H * N], dt)
                    nc.vector.tensor_add(sdd[:], sd_v[:, 0:H * N], sd_v[:, H * N:2 * H * N])
                    mp = pitch(mdiag_rep)
                    nc.vector.tensor_mul(
                        sdd[:], sdd[:], rap(mdiag_rep, 0, [[mp, 128], [0, H], [1, N]])
                    )
                    wdd = small.tile([128, H * N], dt)
                    if LRELU:
                        nc.scalar.activation(wdd[:], sdd[:], AF.Prelu, bias=zbias[:], alpha=SLOPE)
                    else:
                        nc.scalar.activation(wdd[:], sdd[:], AF.Relu, bias=zbias[:])
                    nc.scalar.activation(wdd[:], wdd[:], AF.Exp, bias=zbias[:])
                    rec = small.tile([128, H * N], dt)
                    nc.vector.reciprocal(rec[:], denom[:])
                    attn = attn_pool.tile([128, H * N], dt)
                    nc.vector.tensor_mul(attn[:], wdd[:], rec[:])

                    # ---- prod[f, (h,c,n)] = x[f,(c,n)] * attn[f,(h,n)] ----
                    xt = xts[b][ch]
                    prod = prod_pool.tile([128, H * C * N], dt)
                    xp_, ap_ = pitch(xt), pitch(attn)
                    nc.vector.tensor_mul(
                        prod[:],
                        rap(xt, 0, [[xp_, 128], [0, H], [N, C], [1, N]]),
                        rap(attn, 0, [[ap_, 128], [N, H], [0, C], [1, N]]),
                    )

                    # ---- gather to rhs32 rows (two 64-frame groups stacked)
                    #      and K=32 block-diag matmul -> [128=(mh,k), 512] ----
                    ppr = pitch(prod)
                    R = H * C + H  # 16
                    rhs = rhs_pool.tile([2 * R, 64 * N], dt)
                    for r in range(H * C):
                        nc.sync.dma_start(
                            out=rhs[2 * r:2 * r + 2, :],
                            in_=rap(prod, r * N, [[ppr, 128], [1, N]]),
                        )
                    for h in range(H):
                        r = H * C + h
                        nc.sync.dma_start(
                            out=rhs[2 * r:2 * r + 2, :],
                            in_=rap(attn, h * N, [[ap_, 128], [1, N]]),
                        )
                    for tp in range(4):  # two 8-frame-per-group matmuls each
                        po = psum_o_pool.tile([128, 1024], dt)
                        for t2 in range(2):
                            t = tp * 2 + t2
                            nc.tensor.matmul(
                                po[:, t2 * 512:(t2 + 1) * 512],
                                wout_sb[:], rhs[:, t * 512:(t + 1) * 512],
                                start=True, stop=True,
                            )
                        st = stage_pool.tile([128, 1024], dt)
                        nc.scalar.copy(st[:], po[:])
                        base = b * OS_B + (f0 + tp * 16) * OS_F
                        nc.scalar.dma_start(
                            out=rap(out_c, base,
                                    [[64 * OS_F, 2], [OS_K, H * D], [OS_F, 16], [1, N]]),
                            in_=st[:],
                        )
    nc.compile()
    return nc


def _host_prep(x, mask, W1, b1, W2, b2):
    x = np.ascontiguousarray(np.asarray(x, dtype=np.float32))
    xt = np.ascontiguousarray(x.transpose(0, 1, 3, 2))  # [B, C, N, F]
    mask = np.ascontiguousarray(np.asarray(mask, dtype=np.float32))
    W1 = np.asarray(W1, dtype=np.float32)
    b1 = np.asarray(b1, dtype=np.float32)
    W2 = np.asarray(W2, dtype=np.float32)
    b2 = np.asarray(b2, dtype=np.float32)

    a_src, a_dst = W2[:D, 0], W2[D:, 0]
    W1h = W1.reshape(C, H, D)
    b1h = b1.reshape(H, D)
    mat3 = np.concatenate([W1h @ a_src, W1h @ a_dst], axis=1).astype(np.float32)  # [3, 8]
    v_src = (b1h @ a_src + b2[0]).astype(np.float32)  # [4] (b2 folded here)
    v_dst = (b1h @ a_dst).astype(np.float32)
    vall = np.concatenate([np.repeat(v_src, N), np.repeat(v_dst, N)])[None, :].astype(np.float32)

    # block-delta projection weight: [(c,i'), (hcol, i)] = mat3[c,hcol]*delta(i,i')
    wdel = np.zeros((C, IG, 2 * H, IG), dtype=np.float32)
    for c in range(C):
        for hc in range(2 * H):
            wdel[c, :, hc, :] = mat3[c, hc] * np.eye(IG, dtype=np.float32)
    wdel = wdel.reshape(C * IG, 2 * H * IG)

    wsmall = np.zeros((H * C + H, H * D), dtype=np.float32)
    for h in range(H):
        for c in range(C):
            wsmall[h * C + c, h * D:(h + 1) * D] = W1[c, h * D:(h + 1) * D]
        wsmall[H * C + h, h * D:(h + 1) * D] = b1[h * D:(h + 1) * D]
    # block-diagonal x2 so the out matmul fills all 128 PSUM partitions;
    # the two 32-frame groups (mh) are interleaved as rows 2r+mh
    R = H * C + H
    wout = np.zeros((2 * R, 2 * H * D), dtype=np.float32)
    for mh in range(2):
        wout[mh::2, mh * H * D:(mh + 1) * H * D] = wsmall

    return x, xt, mask, wdel, vall, wout


def _run(inputs, trace=False):
    from concourse.bass_utils import run_bass_kernel_spmd

    x, xt, mask, wdel, vall, wout = _host_prep(
        inputs["x"], inputs["mask"], inputs["W1"], inputs["b1"],
        inputs["W2"], inputs["b2"],
    )
    if "nc" not in _CACHE:
        _CACHE["nc"] = _build_nc()
    nc = _CACHE["nc"]

    in_maps = []
    for c in range(NCORES):
        in_maps.append({
            "xc": np.ascontiguousarray(x[c * BPC:(c + 1) * BPC]),
            "xtc": np.ascontiguousarray(xt[c * BPC:(c + 1) * BPC]),
            "maskf": mask.astype(np.float16),
            "mdiag": np.ascontiguousarray(np.diag(mask))[None, :],
            "wdel": wdel,
            "vall": vall,
            "wout": wout,
        })
    res = run_bass_kernel_spmd(nc, in_maps, core_ids=list(range(NCORES)), trace=trace)
    out = np.concatenate([r["out_c"] for r in res.results], axis=0)
    return out, res


def kernel(**inputs):
    out, _ = _run(inputs, trace=False)
    return out


if __name__ == "__main__":
    rng = np.random.default_rng(0)
    ins = {
        "x": rng.standard_normal((B, C, F, N), dtype=np.float32),
        "mask": rng.random((N, N), dtype=np.float32),
        "W1": 0.1 * rng.standard_normal((C, H * D), dtype=np.float32),
        "b1": 0.1 * rng.standard_normal((H * D,), dtype=np.float32),
        "W2": 0.1 * rng.standard_normal((2 * D, 1), dtype=np.float32),
        "b2": 0.1 * rng.standard_normal((1,), dtype=np.float32),
    }
    out = kernel(**ins)
    print(out.shape, out.dtype)
